# revision 7
# baseline (speedup 1.0000x reference)
"""Distributed Trainium2 kernel for the Ada_GCNResnet block — v4.1 "Gram route".

The N x N adjacency is never materialized:  adj = rinv ⊙ (feat^T feat), so
each GCN layer reassociates as  S @ s1 = feat^T (feat @ s1)  with
M1 = feat @ s1 a D x D matrix.  Nodes are sharded (R = 392 per core); each
core computes the partial  M1p = feat_local @ s1_local,  the partials are
AllReduced (2 MB bf16), and  gc1 = relu(rinv ⊙ (M1^T @ feat_local) + b1)
is fully local.  Same for layer 2.  rinv is computed on the host and
uploaded (1.6 KB/core), then broadcast across partitions with a rank-1
matmul.  Per-core MACs: 4.11G (vs 6.24G for the adjacency route), and the
only collectives are two D x D AllReduces.

v4.1 schedule:
- AR1 is split into 4 column-quarter AllReduces (0.5 MB each) triggered as
  soon as each half of M1p lands; gc1 consumes quarter-by-quarter so the
  PE pipelines with the AR queue.
- AR2 is split into 2 column-half AllReduces; gc2 + the first half of the
  conv1x1-up contraction (k=0..3, accumulated in SBUF fp32) run under
  AR2's second half.
- 3 DMA rings (sync/scalar/vector-triggered) so w1 and late weights
  stream concurrently with the wdT/x input burst; gpsimd stays free to
  trigger collectives.
- all matmuls bf16 + fp32 PSUM (a host-side study measured fp8 DoubleRow
  at ~3.5-5% fro error per converted matmul — over the 2e-2 budget).
"""

import sys

sys.path.insert(0, "/opt/trn_rl_repo")

import numpy as np
import ml_dtypes

from concourse import bacc, tile, mybir
from concourse.bass_utils import run_bass_kernel_spmd

NCORES = 8
B, C, D = 16, 2048, 1024
N = 3136
R = N // NCORES          # 392 local nodes = 2 images
NCLS = 80
KC = C // 128            # 16
KD = D // 128            # 8
RT = [(0, 128), (128, 128), (256, 128), (384, 8)]   # r-tiles of R

F32 = mybir.dt.float32
BF16 = mybir.dt.bfloat16
RG = [list(range(NCORES))]

_BUILT = None


def _build():
    nc = bacc.Bacc("TRN2", target_bir_lowering=False, debug=False,
                   num_devices=NCORES)

    dp = nc.declare_dram_parameter
    xbf_d = dp("xbf", [C, R], BF16, isOutput=False)
    wdT_d = dp("wdT", [C, D], BF16, isOutput=False)
    w1_d = dp("w1", [D, D], BF16, isOutput=False)
    w2_d = dp("w2", [D, D], BF16, isOutput=False)
    wuT_d = dp("wuT", [D, C], BF16, isOutput=False)
    wf_d = dp("wfT", [C, NCLS], F32, isOutput=False)
    bd_d = dp("bd", [D, 1], F32, isOutput=False)
    b1_d = dp("b1", [D, 1], F32, isOutput=False)
    b2_d = dp("b2", [D, 1], F32, isOutput=False)
    bnA_d = dp("bnA", [C, 1], F32, isOutput=False)
    bnB_d = dp("bnB", [C, 1], F32, isOutput=False)
    bfc_d = dp("bfc", [NCLS, 1], F32, isOutput=False)
    rinv_d = dp("rinv", [1, R], F32, isOutput=False)
    id_d = dp("ident", [128, 128], BF16, isOutput=False)
    out_d = dp("out", [NCLS, 2], F32, isOutput=True)

    with tile.TileContext(nc) as tc:
        with (
            tc.tile_pool(name="wpool", bufs=1) as wp,
            tc.tile_pool(name="main", bufs=1) as mp,
            tc.tile_pool(name="dram", bufs=1, space="DRAM") as dr,
        ):
            # ---- long-lived SBUF
            w1_sb = wp.tile([128, KD, D], BF16)
            w2_sb = wp.tile([128, KD, D], BF16)
            wuT_sb = wp.tile([128, KD, C], BF16)
            wf_sb = wp.tile([128, KC, NCLS], F32)
            bd_sb = wp.tile([128, KD], F32)
            b1_sb = wp.tile([128, KD], F32)
            b2_sb = wp.tile([128, KD], F32)
            bnA_sb = wp.tile([128, KC], F32)
            bnB_sb = wp.tile([128, KC], F32)
            bfc_sb = wp.tile([NCLS, 1], F32)
            id_sb = wp.tile([128, 128], BF16)
            ones_sb = wp.tile([1, 128], F32)
            rinv_sb = wp.tile([1, R], F32)

            xbf_sb = mp.tile([128, KC, R], BF16)
            feat_bf = mp.tile([128, KD, R], BF16)
            featT = mp.tile([128, 4, D], BF16)      # [r(part), rt, d]
            gc1T = mp.tile([128, KD, R], BF16)
            s2_bf = mp.tile([128, 4, D], BF16)      # [r(part), rt, d2]
            gc2T = mp.tile([128, KD, R], BF16)
            rinv_bc = mp.tile([128, R], F32)
            pooled = mp.tile([128, KC, 2], F32)
            out_sb = mp.tile([NCLS, 2], F32)

            # ---- DRAM bounce buffers: AR1 in 4 column-quarters, AR2 in 2
            m1b = [dr.tile([D, 256], BF16, name=f"m1b{q}") for q in range(4)]
            m1g = [dr.tile([D, 256], BF16, addr_space="Shared", name=f"m1g{q}")
                   for q in range(4)]
            m2b = [dr.tile([D, 512], BF16, name=f"m2b{h}") for h in range(2)]
            m2g = [dr.tile([D, 512], BF16, addr_space="Shared", name=f"m2g{h}")
                   for h in range(2)]
            m1b_r = [t[:].rearrange("(k p) c -> p k c", p=128) for t in m1b]
            m1g_r = [t[:].rearrange("(k p) c -> p k c", p=128) for t in m1g]
            m2b_r = [t[:].rearrange("(k p) c -> p k c", p=128) for t in m2b]
            m2g_r = [t[:].rearrange("(k p) c -> p k c", p=128) for t in m2g]

            _eng = [nc.sync, nc.scalar]
            _ei = [0]

            def dma(*a, **k):
                e = _eng[_ei[0] % len(_eng)]
                _ei[0] += 1
                return e.dma_start(*a, **k)

            # =========== phase 0+1: input loads and conv1x1-down ===========
            with tc.tile_pool(name="downp", bufs=1) as dnp:
                wdT_sb = dnp.tile([128, KC, D], BF16)

                # ring 3 (vector): everything except the wdT/x burst, in
                # consumption order
                nc.gpsimd.dma_start(
                    bd_sb[:], bd_d.ap().rearrange("(k p) one -> p (k one)", p=128))
                nc.gpsimd.dma_start(id_sb[:], id_d.ap())
                nc.gpsimd.dma_start(rinv_sb[:], rinv_d.ap())
                w1_r = w1_d.ap().rearrange("(k p) d -> p k d", p=128)
                for k in range(KD):
                    nc.gpsimd.dma_start(w1_sb[:, k, :], w1_r[:, k, :])
                nc.gpsimd.dma_start(
                    b1_sb[:], b1_d.ap().rearrange("(k p) one -> p (k one)", p=128))
                w2_r = w2_d.ap().rearrange("(k p) d -> p k d", p=128)
                for k in range(KD):
                    nc.gpsimd.dma_start(w2_sb[:, k, :], w2_r[:, k, :])
                nc.gpsimd.dma_start(
                    b2_sb[:], b2_d.ap().rearrange("(k p) one -> p (k one)", p=128))
                wuT_r = wuT_d.ap().rearrange("(k p) c -> p k c", p=128)
                for k in range(KD):
                    nc.gpsimd.dma_start(wuT_sb[:, k, :], wuT_r[:, k, :])
                nc.gpsimd.dma_start(
                    bnA_sb[:], bnA_d.ap().rearrange("(k p) one -> p (k one)", p=128))
                nc.gpsimd.dma_start(
                    bnB_sb[:], bnB_d.ap().rearrange("(k p) one -> p (k one)", p=128))
                wf_r = wf_d.ap().rearrange("(k p) o -> p k o", p=128)
                for k in range(KC):
                    nc.gpsimd.dma_start(wf_sb[:, k, :], wf_r[:, k, :])
                nc.gpsimd.dma_start(bfc_sb[:], bfc_d.ap())

                # rings 1+2: the down operands, k-interleaved crosswise
                xbf_r = xbf_d.ap().rearrange("(k p) r -> p k r", p=128)
                wdT_r = wdT_d.ap().rearrange("(k p) d -> p k d", p=128)
                for k in range(KC):
                    ea, eb = (nc.sync, nc.scalar) if k % 2 == 0 else (nc.scalar, nc.sync)
                    ea.dma_start(wdT_sb[:, k, :], wdT_r[:, k, :])
                    eb.dma_start(xbf_sb[:, k, :], xbf_r[:, k, :])
                nc.vector.memset(ones_sb[:], 1.0)

                with tc.tile_pool(name="ps0", bufs=1, space="PSUM") as ps0:
                    pds = [ps0.tile([128, R], F32, tag=f"down{m}", name=f"pd{m}")
                           for m in range(KD)]
                    for k in range(KC):
                        for m in range(KD):
                            nc.tensor.matmul(pds[m][:],
                                             wdT_sb[:, k, 128 * m:128 * (m + 1)],
                                             xbf_sb[:, k, :],
                                             start=(k == 0), stop=(k == KC - 1))
                    for m in range(KD):
                        nc.vector.tensor_scalar_add(feat_bf[:, m, :], pds[m][:],
                                                    bd_sb[:, m:m + 1])

            # ===== phase 2: transposes, s1 (r-major), M1 partial, AR1 =====
            with (
                tc.tile_pool(name="s1pool", bufs=1) as s1p,
                tc.tile_pool(name="ps1", bufs=1, space="PSUM") as ps1,
            ):
                s1_rm = s1p.tile([128, 4, D], BF16)   # [r(part), rt, d1]

                for m in range(KD):
                    for t, (rof, wt) in enumerate(RT):
                        pt = ps1.tile([128, 128], BF16, tag="tr", bufs=2,
                                      name=f"ptr{m}_{t}")
                        nc.tensor.transpose(pt[:wt, :],
                                            feat_bf[:, m, rof:rof + wt],
                                            id_sb[:])
                        nc.vector.tensor_copy(
                            featT[:wt, t, 128 * m:128 * (m + 1)], pt[:wt, :])

                for h in range(2):
                    for t, (rof, wt) in enumerate(RT):
                        p = ps1.tile([128, 512], F32, tag=f"s1_{t}",
                                     name=f"ps1_{h}_{t}")
                        for k in range(KD):
                            nc.tensor.matmul(
                                p[:wt], feat_bf[:, k, rof:rof + wt],
                                w1_sb[:, k, 512 * h:512 * (h + 1)],
                                start=(k == 0), stop=(k == KD - 1))
                        nc.vector.tensor_copy(
                            s1_rm[:wt, t, 512 * h:512 * (h + 1)], p[:wt])
                    for m in range(KD):
                        pm = ps1.tile([128, 512], F32, tag="m1p", bufs=2,
                                      name=f"pm1_{h}_{m}")
                        for t, (rof, wt) in enumerate(RT):
                            nc.tensor.matmul(
                                pm[:], featT[:wt, t, 128 * m:128 * (m + 1)],
                                s1_rm[:wt, t, 512 * h:512 * (h + 1)],
                                start=(t == 0), stop=(t == 3))
                        with tc.high_priority(offset=5000):
                            ev = s1p.tile([128, 512], BF16, tag="m1e", bufs=3,
                                          name=f"m1e_{h}_{m}")
                            nc.vector.tensor_copy(ev[:], pm[:])
                            nc.sync.dma_start(m1b_r[2 * h][:, m, :], ev[:, :256])
                            nc.sync.dma_start(m1b_r[2 * h + 1][:, m, :], ev[:, 256:])
                    with tc.high_priority(offset=5000):
                        for q in (2 * h, 2 * h + 1):
                            nc.gpsimd.collective_compute(
                                "AllReduce", mybir.AluOpType.add,
                                replica_groups=RG,
                                ins=[m1b[q][:].opt()], outs=[m1g[q][:].opt()])

            # ===== phase 3: gc1 = relu(rinv*(M1^T feat)+b1), per quarter ===
            with (
                tc.tile_pool(name="m1pool", bufs=1) as m1p_,
                tc.tile_pool(name="ps2", bufs=1, space="PSUM") as ps2,
            ):
                m1_sb = m1p_.tile([128, KD, D], BF16)

                # rinv broadcast across partitions (rank-1 matmul)
                prb = ps2.tile([128, R], F32, tag="rbc")
                nc.tensor.matmul(prb[:], ones_sb[:], rinv_sb[:],
                                 start=True, stop=True)
                nc.vector.tensor_copy(rinv_bc[:], prb[:])

                for q in range(4):
                    with tc.high_priority(offset=5000):
                        for k in range(KD):
                            dma(m1_sb[:, k, 256 * q:256 * (q + 1)],
                                m1g_r[q][:, k, :])
                    pz = [ps2.tile([128, R], F32, tag=f"z{q % 2}_{i}",
                                   name=f"pz1_{q}_{i}") for i in range(2)]
                    for k in range(KD):
                        for i in range(2):
                            j = 2 * q + i
                            nc.tensor.matmul(pz[i][:],
                                             m1_sb[:, k, 128 * j:128 * (j + 1)],
                                             feat_bf[:, k, :],
                                             start=(k == 0), stop=(k == KD - 1))
                    for i in range(2):
                        j = 2 * q + i
                        zt = m1p_.tile([128, R], F32, tag="ztmp", bufs=2,
                                       name=f"zt1_{j}")
                        nc.vector.tensor_tensor(zt[:], pz[i][:], rinv_bc[:],
                                                op=mybir.AluOpType.mult)
                        nc.scalar.activation(gc1T[:, j, :], zt[:],
                                             mybir.ActivationFunctionType.Relu,
                                             bias=b1_sb[:, j:j + 1], scale=1.0)

            # ====== phase 4: s2 (r-major), M2 partial, AR2 halves ==========
            with tc.tile_pool(name="ps3", bufs=1, space="PSUM") as ps3:
                for h in range(2):
                    for t, (rof, wt) in enumerate(RT):
                        p = ps3.tile([128, 512], F32, tag="s2", bufs=3,
                                     name=f"ps2_{h}_{t}")
                        for k in range(KD):
                            nc.tensor.matmul(
                                p[:wt], gc1T[:, k, rof:rof + wt],
                                w2_sb[:, k, 512 * h:512 * (h + 1)],
                                start=(k == 0), stop=(k == KD - 1))
                        nc.vector.tensor_copy(
                            s2_bf[:wt, t, 512 * h:512 * (h + 1)], p[:wt])
                for h in range(2):
                    for m in range(KD):
                        pm = ps3.tile([128, 512], F32, tag="m2p", bufs=2,
                                      name=f"pm2_{h}_{m}")
                        for t, (rof, wt) in enumerate(RT):
                            nc.tensor.matmul(
                                pm[:], featT[:wt, t, 128 * m:128 * (m + 1)],
                                s2_bf[:wt, t, 512 * h:512 * (h + 1)],
                                start=(t == 0), stop=(t == 3))
                        with tc.high_priority(offset=5000):
                            ev = mp.tile([128, 512], BF16, tag="m2e", bufs=3,
                                         name=f"m2e_{h}_{m}")
                            nc.vector.tensor_copy(ev[:], pm[:])
                            nc.sync.dma_start(m2b_r[h][:, m, :], ev[:])
                    with tc.high_priority(offset=5000):
                        nc.gpsimd.collective_compute(
                            "AllReduce", mybir.AluOpType.add, replica_groups=RG,
                            ins=[m2b[h][:].opt()], outs=[m2g[h][:].opt()])

            # == phase 5: gc2 + conv-up (k-halves, SBUF accum) + pool + fc ==
            with (
                tc.tile_pool(name="m2pool", bufs=1) as m2p_,
                tc.tile_pool(name="ps4", bufs=1, space="PSUM") as ps4,
            ):
                m2_sb = m2p_.tile([128, KD, D], BF16)
                xup_sb = m2p_.tile([128, KC, R], F32)

                for h in range(2):
                    with tc.high_priority(offset=5000):
                        for k in range(KD):
                            dma(m2_sb[:, k, 512 * h:512 * (h + 1)],
                                m2g_r[h][:, k, :])
                    pz = [ps4.tile([128, R], F32, tag=f"zz{i}",
                                   name=f"pz2_{h}_{i}") for i in range(4)]
                    for k in range(KD):
                        for i in range(4):
                            j = 4 * h + i
                            nc.tensor.matmul(pz[i][:],
                                             m2_sb[:, k, 128 * j:128 * (j + 1)],
                                             feat_bf[:, k, :],
                                             start=(k == 0), stop=(k == KD - 1))
                    for i in range(4):
                        j = 4 * h + i
                        zt = m2p_.tile([128, R], F32, tag="ztmp", bufs=2,
                                       name=f"zt2_{j}")
                        nc.vector.tensor_tensor(zt[:], pz[i][:], rinv_bc[:],
                                                op=mybir.AluOpType.mult)
                        nc.scalar.activation(gc2T[:, j, :], zt[:],
                                             mybir.ActivationFunctionType.Relu,
                                             bias=b2_sb[:, j:j + 1], scale=1.0)

                    # conv-up partial over this half's k-tiles; h=0 runs
                    # under AR2's second half
                    for m in range(KC):
                        pu = ps4.tile([128, R], F32, tag="up", bufs=3)
                        for kk in range(4):
                            k = 4 * h + kk
                            nc.tensor.matmul(pu[:],
                                             wuT_sb[:, k, 128 * m:128 * (m + 1)],
                                             gc2T[:, k, :],
                                             start=(kk == 0), stop=(kk == 3))
                        if h == 0:
                            nc.vector.tensor_copy(xup_sb[:, m, :], pu[:])
                        else:
                            xq = m2p_.tile([128, R], F32, tag="xq", bufs=3)
                            nc.vector.tensor_tensor(xq[:], pu[:],
                                                    xup_sb[:, m, :],
                                                    op=mybir.AluOpType.add)
                            xb = m2p_.tile([128, R], F32, tag="xbn", bufs=3)
                            nc.scalar.activation(
                                xb[:], xq[:],
                                mybir.ActivationFunctionType.Identity,
                                bias=bnB_sb[:, m:m + 1],
                                scale=bnA_sb[:, m:m + 1])
                            xr = m2p_.tile([128, R], F32, tag="xres", bufs=3)
                            nc.vector.tensor_tensor(xr[:], xb[:],
                                                    xbf_sb[:, m, :],
                                                    op=mybir.AluOpType.add)
                            nc.vector.tensor_reduce(
                                pooled[:, m, :],
                                xr[:].rearrange("p (i q) -> p i q", i=2),
                                axis=mybir.AxisListType.X,
                                op=mybir.AluOpType.max)

                with tc.tile_pool(name="ps5", bufs=1, space="PSUM") as ps5:
                    pfc = ps5.tile([NCLS, 2], F32, tag="fc")
                    for k in range(KC):
                        nc.tensor.matmul(pfc[:], wf_sb[:, k, :], pooled[:, k, :],
                                         start=(k == 0), stop=(k == KC - 1))
                    nc.scalar.activation(out_sb[:], pfc[:],
                                         mybir.ActivationFunctionType.Identity,
                                         bias=bfc_sb[:], scale=1.0)
                    dma(out_d[:], out_sb[:])

    nc.compile()
    return nc


def _prep(inputs):
    bf = ml_dtypes.bfloat16
    f = np.ascontiguousarray(inputs["feature"], dtype=np.float32)
    X = np.ascontiguousarray(f.transpose(1, 0, 2, 3).reshape(C, N))

    wdT = np.ascontiguousarray(np.asarray(inputs["w_down"], np.float32).T)
    bd = np.asarray(inputs["b_down"], np.float32)

    # host-exact rinv: feat in fp32 BLAS, rowsum in fp64
    feat32 = wdT.T @ X + bd[:, None]
    f64 = feat32.astype(np.float64)
    rowsum = f64.T @ f64.sum(1) + 1e-10
    rinv = (1.0 / rowsum).astype(np.float32)

    A = (inputs["bn_gamma"] / np.sqrt(inputs["bn_var"] + 1e-5)).astype(np.float32)
    Bb = (inputs["bn_beta"] + (inputs["b_up"] - inputs["bn_mean"]) * A).astype(np.float32)

    com = {
        "wdT": wdT.astype(bf),
        "w1": np.ascontiguousarray(inputs["w1"], dtype=np.float32).astype(bf),
        "w2": np.ascontiguousarray(inputs["w2"], dtype=np.float32).astype(bf),
        "wuT": np.ascontiguousarray(np.asarray(inputs["w_up"], np.float32).T).astype(bf),
        "wfT": np.ascontiguousarray(np.asarray(inputs["w_fc"], np.float32).T),
        "bd": bd.reshape(D, 1),
        "b1": np.asarray(inputs["b1"], np.float32).reshape(D, 1),
        "b2": np.asarray(inputs["b2"], np.float32).reshape(D, 1),
        "bnA": A.reshape(C, 1),
        "bnB": Bb.reshape(C, 1),
        "bfc": np.asarray(inputs["b_fc"], np.float32).reshape(NCLS, 1),
        "ident": np.eye(128, dtype=bf),
    }
    in_maps = []
    for c in range(NCORES):
        m = dict(com)
        m["xbf"] = np.ascontiguousarray(X[:, R * c:R * (c + 1)]).astype(bf)
        m["rinv"] = np.ascontiguousarray(rinv[None, R * c:R * (c + 1)])
        in_maps.append(m)
    return in_maps


def kernel(**inputs):
    global _BUILT
    if _BUILT is None:
        _BUILT = _build()
    in_maps = _prep(inputs)
    res = run_bass_kernel_spmd(_BUILT, in_maps, core_ids=list(range(NCORES)))
    out = np.empty((B, NCLS), dtype=np.float32)
    for c in range(NCORES):
        o = res.results[c]["out"]  # (NCLS, 2)
        out[2 * c] = o[:, 0]
        out[2 * c + 1] = o[:, 1]
    return out


# revision 8
# speedup vs baseline: 1.1716x; 1.1716x over previous
"""Distributed Trainium2 kernel for the Ada_GCNResnet block — v4.1 "Gram route".

The N x N adjacency is never materialized:  adj = rinv ⊙ (feat^T feat), so
each GCN layer reassociates as  S @ s1 = feat^T (feat @ s1)  with
M1 = feat @ s1 a D x D matrix.  Nodes are sharded (R = 392 per core); each
core computes the partial  M1p = feat_local @ s1_local,  the partials are
AllReduced (2 MB bf16), and  gc1 = relu(rinv ⊙ (M1^T @ feat_local) + b1)
is fully local.  Same for layer 2.  rinv is computed on the host and
uploaded (1.6 KB/core), then broadcast across partitions with a rank-1
matmul.  Per-core MACs: 4.11G (vs 6.24G for the adjacency route), and the
only collectives are two D x D AllReduces.

v4.1 schedule:
- AR1 is split into 4 column-quarter AllReduces (0.5 MB each) triggered as
  soon as each half of M1p lands; gc1 consumes quarter-by-quarter so the
  PE pipelines with the AR queue.
- AR2 is split into 2 column-half AllReduces; gc2 + the first half of the
  conv1x1-up contraction (k=0..3, accumulated in SBUF fp32) run under
  AR2's second half.
- 3 DMA rings (sync/scalar/vector-triggered) so w1 and late weights
  stream concurrently with the wdT/x input burst; gpsimd stays free to
  trigger collectives.
- all matmuls bf16 + fp32 PSUM (a host-side study measured fp8 DoubleRow
  at ~3.5-5% fro error per converted matmul — over the 2e-2 budget).
"""

import sys

sys.path.insert(0, "/opt/trn_rl_repo")

import numpy as np
import ml_dtypes

from concourse import bacc, tile, mybir
from concourse.bass_utils import run_bass_kernel_spmd

NCORES = 8
B, C, D = 16, 2048, 1024
N = 3136
R = N // NCORES          # 392 local nodes = 2 images
NCLS = 80
KC = C // 128            # 16
KD = D // 128            # 8
RT = [(0, 128), (128, 128), (256, 128), (384, 8)]   # r-tiles of R

F32 = mybir.dt.float32
BF16 = mybir.dt.bfloat16
RG = [list(range(NCORES))]

_BUILT = None


def _build():
    nc = bacc.Bacc("TRN2", target_bir_lowering=False, debug=False,
                   num_devices=NCORES)

    dp = nc.declare_dram_parameter
    xbf_d = dp("xbf", [C, R], BF16, isOutput=False)
    wdT_d = dp("wdT", [C, D], BF16, isOutput=False)
    w1_d = dp("w1", [D, D], BF16, isOutput=False)
    w2_d = dp("w2", [D, D], BF16, isOutput=False)
    wuT_d = dp("wuT", [D, C], BF16, isOutput=False)
    wf_d = dp("wfT", [C, NCLS], F32, isOutput=False)
    bd_d = dp("bd", [D, 1], F32, isOutput=False)
    b1_d = dp("b1", [D, 1], F32, isOutput=False)
    b2_d = dp("b2", [D, 1], F32, isOutput=False)
    bnA_d = dp("bnA", [C, 1], F32, isOutput=False)
    bnB_d = dp("bnB", [C, 1], F32, isOutput=False)
    bfc_d = dp("bfc", [NCLS, 1], F32, isOutput=False)
    rinv_d = dp("rinv", [1, R], F32, isOutput=False)
    id_d = dp("ident", [128, 128], BF16, isOutput=False)
    out_d = dp("out", [NCLS, 2], F32, isOutput=True)

    with tile.TileContext(nc) as tc:
        with (
            tc.tile_pool(name="wpool", bufs=1) as wp,
            tc.tile_pool(name="main", bufs=1) as mp,
            tc.tile_pool(name="dram", bufs=1, space="DRAM") as dr,
        ):
            # ---- long-lived SBUF
            w1_sb = wp.tile([128, KD, D], BF16)
            w2_sb = wp.tile([128, KD, D], BF16)
            wuT_sb = wp.tile([128, KD, C], BF16)
            wf_sb = wp.tile([128, KC, NCLS], F32)
            bd_sb = wp.tile([128, KD], F32)
            b1_sb = wp.tile([128, KD], F32)
            b2_sb = wp.tile([128, KD], F32)
            bnA_sb = wp.tile([128, KC], F32)
            bnB_sb = wp.tile([128, KC], F32)
            bfc_sb = wp.tile([NCLS, 1], F32)
            id_sb = wp.tile([128, 128], BF16)
            ones_sb = wp.tile([1, 128], F32)
            rinv_sb = wp.tile([1, R], F32)

            xbf_sb = mp.tile([128, KC, R], BF16)
            feat_bf = mp.tile([128, KD, R], BF16)
            featT = mp.tile([128, 4, D], BF16)      # [r(part), rt, d]
            gc1T = mp.tile([128, KD, R], BF16)
            s2_bf = mp.tile([128, 4, D], BF16)      # [r(part), rt, d2]
            gc2T = mp.tile([128, KD, R], BF16)
            rinv_bc = mp.tile([128, R], F32)
            pooled = mp.tile([128, KC, 2], F32)
            out_sb = mp.tile([NCLS, 2], F32)

            # ---- DRAM bounce buffers: AR1 in 4 column-quarters, AR2 in 2
            m1b = [dr.tile([D, 512], BF16, name=f"m1b{q}") for q in range(2)]
            m1g = [dr.tile([D, 512], BF16, addr_space="Shared", name=f"m1g{q}")
                   for q in range(2)]
            dum_b = dr.tile([1, 16], F32)
            dum_g = dr.tile([NCORES, 16], F32, addr_space="Shared")
            m2b = [dr.tile([D, 512], BF16, name=f"m2b{h}") for h in range(2)]
            m2g = [dr.tile([D, 512], BF16, addr_space="Shared", name=f"m2g{h}")
                   for h in range(2)]
            m1b_r = [t[:].rearrange("(k p) c -> p k c", p=128) for t in m1b]
            m1g_r = [t[:].rearrange("(k p) c -> p k c", p=128) for t in m1g]
            m2b_r = [t[:].rearrange("(k p) c -> p k c", p=128) for t in m2b]
            m2g_r = [t[:].rearrange("(k p) c -> p k c", p=128) for t in m2g]

            _eng = [nc.sync, nc.scalar]
            _ei = [0]

            def dma(*a, **k):
                e = _eng[_ei[0] % len(_eng)]
                _ei[0] += 1
                return e.dma_start(*a, **k)

            # =========== phase 0+1: input loads and conv1x1-down ===========
            with tc.tile_pool(name="downp", bufs=1) as dnp:
                wdT_sb = dnp.tile([128, KC, D], BF16)

                # tiny dummy collective, first thing: absorbs the ~40us
                # cross-core entry barrier and the ~13us first-op setup
                # under the down phase
                with tc.high_priority(offset=9000):
                    nc.vector.memset(ones_sb[:], 1.0)
                    nc.sync.dma_start(dum_b[:], ones_sb[:1, :16])
                    nc.gpsimd.collective_compute(
                        "AllGather", mybir.AluOpType.bypass, replica_groups=RG,
                        ins=[dum_b[:].opt()], outs=[dum_g[:].opt()])

                # ring 3 (vector): everything except the wdT/x burst, in
                # consumption order
                nc.gpsimd.dma_start(
                    bd_sb[:], bd_d.ap().rearrange("(k p) one -> p (k one)", p=128))
                nc.gpsimd.dma_start(id_sb[:], id_d.ap())
                nc.gpsimd.dma_start(rinv_sb[:], rinv_d.ap())
                w1_r = w1_d.ap().rearrange("(k p) d -> p k d", p=128)
                for k in range(KD):
                    nc.gpsimd.dma_start(w1_sb[:, k, :], w1_r[:, k, :])
                nc.gpsimd.dma_start(
                    b1_sb[:], b1_d.ap().rearrange("(k p) one -> p (k one)", p=128))

                # rings 1+2: the down operands, k-interleaved crosswise
                xbf_r = xbf_d.ap().rearrange("(k p) r -> p k r", p=128)
                wdT_r = wdT_d.ap().rearrange("(k p) d -> p k d", p=128)
                for k in range(KC):
                    ea, eb = (nc.sync, nc.scalar) if k % 2 == 0 else (nc.scalar, nc.sync)
                    ea.dma_start(wdT_sb[:, k, :], wdT_r[:, k, :])
                    eb.dma_start(xbf_sb[:, k, :], xbf_r[:, k, :])

                # late weights after the burst, on the gpsimd ring
                w2_r = w2_d.ap().rearrange("(k p) d -> p k d", p=128)
                for k in range(KD):
                    nc.gpsimd.dma_start(w2_sb[:, k, :], w2_r[:, k, :])
                nc.gpsimd.dma_start(
                    b2_sb[:], b2_d.ap().rearrange("(k p) one -> p (k one)", p=128))
                wuT_r = wuT_d.ap().rearrange("(k p) c -> p k c", p=128)
                for k in range(KD):
                    nc.gpsimd.dma_start(wuT_sb[:, k, :], wuT_r[:, k, :])
                nc.gpsimd.dma_start(
                    bnA_sb[:], bnA_d.ap().rearrange("(k p) one -> p (k one)", p=128))
                nc.gpsimd.dma_start(
                    bnB_sb[:], bnB_d.ap().rearrange("(k p) one -> p (k one)", p=128))
                wf_r = wf_d.ap().rearrange("(k p) o -> p k o", p=128)
                for k in range(KC):
                    nc.gpsimd.dma_start(wf_sb[:, k, :], wf_r[:, k, :])
                nc.gpsimd.dma_start(bfc_sb[:], bfc_d.ap())

                with tc.tile_pool(name="ps0", bufs=1, space="PSUM") as ps0:
                    pds = [ps0.tile([128, R], F32, tag=f"down{m}", name=f"pd{m}")
                           for m in range(KD)]
                    for k in range(KC):
                        for m in range(KD):
                            nc.tensor.matmul(pds[m][:],
                                             wdT_sb[:, k, 128 * m:128 * (m + 1)],
                                             xbf_sb[:, k, :],
                                             start=(k == 0), stop=(k == KC - 1))
                    for m in range(KD):
                        nc.vector.tensor_scalar_add(feat_bf[:, m, :], pds[m][:],
                                                    bd_sb[:, m:m + 1])

            # ===== phase 2: transposes, s1 (r-major), M1 partial, AR1 =====
            with (
                tc.tile_pool(name="s1pool", bufs=1) as s1p,
                tc.tile_pool(name="ps1", bufs=1, space="PSUM") as ps1,
            ):
                s1_rm = s1p.tile([128, 4, D], BF16)   # [r(part), rt, d1]

                for m in range(KD):
                    for t, (rof, wt) in enumerate(RT):
                        pt = ps1.tile([128, 128], BF16, tag="tr", bufs=2,
                                      name=f"ptr{m}_{t}")
                        nc.tensor.transpose(pt[:wt, :],
                                            feat_bf[:, m, rof:rof + wt],
                                            id_sb[:])
                        nc.vector.tensor_copy(
                            featT[:wt, t, 128 * m:128 * (m + 1)], pt[:wt, :])

                for h in range(2):
                    for t, (rof, wt) in enumerate(RT):
                        p = ps1.tile([128, 512], F32, tag=f"s1_{t}",
                                     name=f"ps1_{h}_{t}")
                        for k in range(KD):
                            nc.tensor.matmul(
                                p[:wt], feat_bf[:, k, rof:rof + wt],
                                w1_sb[:, k, 512 * h:512 * (h + 1)],
                                start=(k == 0), stop=(k == KD - 1))
                        nc.vector.tensor_copy(
                            s1_rm[:wt, t, 512 * h:512 * (h + 1)], p[:wt])
                    for m in range(KD):
                        pm = ps1.tile([128, 512], F32, tag="m1p", bufs=2,
                                      name=f"pm1_{h}_{m}")
                        for t, (rof, wt) in enumerate(RT):
                            nc.tensor.matmul(
                                pm[:], featT[:wt, t, 128 * m:128 * (m + 1)],
                                s1_rm[:wt, t, 512 * h:512 * (h + 1)],
                                start=(t == 0), stop=(t == 3))
                        with tc.high_priority(offset=5000):
                            ev = s1p.tile([128, 512], BF16, tag="m1e", bufs=3,
                                          name=f"m1e_{h}_{m}")
                            nc.vector.tensor_copy(ev[:], pm[:])
                            nc.sync.dma_start(m1b_r[h][:, m, :], ev[:])
                    with tc.high_priority(offset=5000):
                        nc.gpsimd.collective_compute(
                            "AllReduce", mybir.AluOpType.add, replica_groups=RG,
                            ins=[m1b[h][:].opt()], outs=[m1g[h][:].opt()])

            # ===== phase 3: gc1 = relu(rinv*(M1^T feat)+b1), per quarter ===
            with (
                tc.tile_pool(name="m1pool", bufs=1) as m1p_,
                tc.tile_pool(name="ps2", bufs=1, space="PSUM") as ps2,
            ):
                m1_sb = m1p_.tile([128, KD, D], BF16)

                # rinv broadcast across partitions (rank-1 matmul)
                prb = ps2.tile([128, R], F32, tag="rbc")
                nc.tensor.matmul(prb[:], ones_sb[:], rinv_sb[:],
                                 start=True, stop=True)
                nc.vector.tensor_copy(rinv_bc[:], prb[:])

                for hh in range(2):
                    with tc.high_priority(offset=5000):
                        for k in range(KD):
                            dma(m1_sb[:, k, 512 * hh:512 * (hh + 1)],
                                m1g_r[hh][:, k, :])
                    pz = [ps2.tile([128, R], F32, tag=f"z{i}",
                                   name=f"pz1_{hh}_{i}") for i in range(4)]
                    for k in range(KD):
                        for i in range(4):
                            j = 4 * hh + i
                            nc.tensor.matmul(pz[i][:],
                                             m1_sb[:, k, 128 * j:128 * (j + 1)],
                                             feat_bf[:, k, :],
                                             start=(k == 0), stop=(k == KD - 1))
                    for i in range(4):
                        j = 4 * hh + i
                        zt = m1p_.tile([128, R], F32, tag="ztmp", bufs=2,
                                       name=f"zt1_{j}")
                        nc.vector.tensor_tensor(zt[:], pz[i][:], rinv_bc[:],
                                                op=mybir.AluOpType.mult)
                        nc.scalar.activation(gc1T[:, j, :], zt[:],
                                             mybir.ActivationFunctionType.Relu,
                                             bias=b1_sb[:, j:j + 1], scale=1.0)

            # ====== phase 4: s2 (r-major), M2 partial, AR2 halves ==========
            with tc.tile_pool(name="ps3", bufs=1, space="PSUM") as ps3:
                for h in range(2):
                    for t, (rof, wt) in enumerate(RT):
                        p = ps3.tile([128, 512], F32, tag="s2", bufs=3,
                                     name=f"ps2_{h}_{t}")
                        for k in range(KD):
                            nc.tensor.matmul(
                                p[:wt], gc1T[:, k, rof:rof + wt],
                                w2_sb[:, k, 512 * h:512 * (h + 1)],
                                start=(k == 0), stop=(k == KD - 1))
                        nc.vector.tensor_copy(
                            s2_bf[:wt, t, 512 * h:512 * (h + 1)], p[:wt])
                    for m in range(KD):
                        pm = ps3.tile([128, 512], F32, tag="m2p", bufs=2,
                                      name=f"pm2_{h}_{m}")
                        for t, (rof, wt) in enumerate(RT):
                            nc.tensor.matmul(
                                pm[:], featT[:wt, t, 128 * m:128 * (m + 1)],
                                s2_bf[:wt, t, 512 * h:512 * (h + 1)],
                                start=(t == 0), stop=(t == 3))
                        with tc.high_priority(offset=5000):
                            ev = mp.tile([128, 512], BF16, tag="m2e", bufs=3,
                                         name=f"m2e_{h}_{m}")
                            nc.vector.tensor_copy(ev[:], pm[:])
                            nc.sync.dma_start(m2b_r[h][:, m, :], ev[:])
                    with tc.high_priority(offset=5000):
                        nc.gpsimd.collective_compute(
                            "AllReduce", mybir.AluOpType.add, replica_groups=RG,
                            ins=[m2b[h][:].opt()], outs=[m2g[h][:].opt()])

            # == phase 5: gc2 + conv-up (k-halves, SBUF accum) + pool + fc ==
            with (
                tc.tile_pool(name="m2pool", bufs=1) as m2p_,
                tc.tile_pool(name="ps4", bufs=1, space="PSUM") as ps4,
            ):
                m2_sb = m2p_.tile([128, KD, D], BF16)
                xup_sb = m2p_.tile([128, KC, R], F32)

                for h in range(2):
                    with tc.high_priority(offset=5000):
                        for k in range(KD):
                            dma(m2_sb[:, k, 512 * h:512 * (h + 1)],
                                m2g_r[h][:, k, :])
                    pz = [ps4.tile([128, R], F32, tag=f"zz{i}",
                                   name=f"pz2_{h}_{i}") for i in range(4)]
                    for k in range(KD):
                        for i in range(4):
                            j = 4 * h + i
                            nc.tensor.matmul(pz[i][:],
                                             m2_sb[:, k, 128 * j:128 * (j + 1)],
                                             feat_bf[:, k, :],
                                             start=(k == 0), stop=(k == KD - 1))
                    for i in range(4):
                        j = 4 * h + i
                        zt = m2p_.tile([128, R], F32, tag="ztmp", bufs=2,
                                       name=f"zt2_{j}")
                        nc.vector.tensor_tensor(zt[:], pz[i][:], rinv_bc[:],
                                                op=mybir.AluOpType.mult)
                        nc.scalar.activation(gc2T[:, j, :], zt[:],
                                             mybir.ActivationFunctionType.Relu,
                                             bias=b2_sb[:, j:j + 1], scale=1.0)

                    # conv-up partial over this half's k-tiles; h=0 runs
                    # under AR2's second half
                    for m in range(KC):
                        pu = ps4.tile([128, R], F32, tag="up", bufs=3)
                        for kk in range(4):
                            k = 4 * h + kk
                            nc.tensor.matmul(pu[:],
                                             wuT_sb[:, k, 128 * m:128 * (m + 1)],
                                             gc2T[:, k, :],
                                             start=(kk == 0), stop=(kk == 3))
                        if h == 0:
                            nc.vector.tensor_copy(xup_sb[:, m, :], pu[:])
                        else:
                            xq = m2p_.tile([128, R], F32, tag="xq", bufs=3)
                            nc.vector.tensor_tensor(xq[:], pu[:],
                                                    xup_sb[:, m, :],
                                                    op=mybir.AluOpType.add)
                            xb = m2p_.tile([128, R], F32, tag="xbn", bufs=3)
                            nc.scalar.activation(
                                xb[:], xq[:],
                                mybir.ActivationFunctionType.Identity,
                                bias=bnB_sb[:, m:m + 1],
                                scale=bnA_sb[:, m:m + 1])
                            xr = m2p_.tile([128, R], F32, tag="xres", bufs=3)
                            nc.vector.tensor_tensor(xr[:], xb[:],
                                                    xbf_sb[:, m, :],
                                                    op=mybir.AluOpType.add)
                            nc.vector.tensor_reduce(
                                pooled[:, m, :],
                                xr[:].rearrange("p (i q) -> p i q", i=2),
                                axis=mybir.AxisListType.X,
                                op=mybir.AluOpType.max)

                with tc.tile_pool(name="ps5", bufs=1, space="PSUM") as ps5:
                    pfc = ps5.tile([NCLS, 2], F32, tag="fc")
                    for k in range(KC):
                        nc.tensor.matmul(pfc[:], wf_sb[:, k, :], pooled[:, k, :],
                                         start=(k == 0), stop=(k == KC - 1))
                    nc.scalar.activation(out_sb[:], pfc[:],
                                         mybir.ActivationFunctionType.Identity,
                                         bias=bfc_sb[:], scale=1.0)
                    dma(out_d[:], out_sb[:])

    nc.compile()
    return nc


def _prep(inputs):
    bf = ml_dtypes.bfloat16
    f = np.ascontiguousarray(inputs["feature"], dtype=np.float32)
    X = np.ascontiguousarray(f.transpose(1, 0, 2, 3).reshape(C, N))

    wdT = np.ascontiguousarray(np.asarray(inputs["w_down"], np.float32).T)
    bd = np.asarray(inputs["b_down"], np.float32)

    # host-exact rinv: feat in fp32 BLAS, rowsum in fp64
    feat32 = wdT.T @ X + bd[:, None]
    f64 = feat32.astype(np.float64)
    rowsum = f64.T @ f64.sum(1) + 1e-10
    rinv = (1.0 / rowsum).astype(np.float32)

    A = (inputs["bn_gamma"] / np.sqrt(inputs["bn_var"] + 1e-5)).astype(np.float32)
    Bb = (inputs["bn_beta"] + (inputs["b_up"] - inputs["bn_mean"]) * A).astype(np.float32)

    com = {
        "wdT": wdT.astype(bf),
        "w1": np.ascontiguousarray(inputs["w1"], dtype=np.float32).astype(bf),
        "w2": np.ascontiguousarray(inputs["w2"], dtype=np.float32).astype(bf),
        "wuT": np.ascontiguousarray(np.asarray(inputs["w_up"], np.float32).T).astype(bf),
        "wfT": np.ascontiguousarray(np.asarray(inputs["w_fc"], np.float32).T),
        "bd": bd.reshape(D, 1),
        "b1": np.asarray(inputs["b1"], np.float32).reshape(D, 1),
        "b2": np.asarray(inputs["b2"], np.float32).reshape(D, 1),
        "bnA": A.reshape(C, 1),
        "bnB": Bb.reshape(C, 1),
        "bfc": np.asarray(inputs["b_fc"], np.float32).reshape(NCLS, 1),
        "ident": np.eye(128, dtype=bf),
    }
    in_maps = []
    for c in range(NCORES):
        m = dict(com)
        m["xbf"] = np.ascontiguousarray(X[:, R * c:R * (c + 1)]).astype(bf)
        m["rinv"] = np.ascontiguousarray(rinv[None, R * c:R * (c + 1)])
        in_maps.append(m)
    return in_maps


def kernel(**inputs):
    global _BUILT
    if _BUILT is None:
        _BUILT = _build()
    in_maps = _prep(inputs)
    res = run_bass_kernel_spmd(_BUILT, in_maps, core_ids=list(range(NCORES)))
    out = np.empty((B, NCLS), dtype=np.float32)
    for c in range(NCORES):
        o = res.results[c]["out"]  # (NCLS, 2)
        out[2 * c] = o[:, 0]
        out[2 * c + 1] = o[:, 1]
    return out
